# revision 1
# baseline (speedup 1.0000x reference)
"""Multi-head attention (B=4, T=2048, D=1024, H=16) on 8 TRN2 NeuronCores.

Sharding: core c handles batch b = c//2 and head-half hh = c%2 (8 heads,
512 of the 1024 channel dims). Each core computes its half of the head
outputs and a row-sharded output projection, producing a partial
[T, D] output. Host unshard: out[b] = partial[2b] + partial[2b+1]
+ b_o + b_v @ w_o.T (the value-bias contribution commutes through
attention because softmax rows sum to 1).

All matmuls run in float32r (hi/lo split on the PE at full rate,
rms rel err ~1.5e-4 per matmul).
"""

from contextlib import ExitStack

import numpy as np

import concourse.bass as bass
import concourse.mybir as mybir
import concourse.tile as tile
from concourse import bacc
from concourse.bass_utils import run_bass_kernel_spmd

B, T, D = 4, 2048, 1024
H = 16
DH = 64  # head dim
HALF = 512  # channels per core (8 heads)
N_CORES = 8

F32 = mybir.dt.float32
F32R = mybir.dt.float32r

TB = 512  # t-block for moving operands
NTB = T // TB  # 4
KB = 128  # contraction block
NKB = D // KB  # 8
NJB = HALF // KB  # 4 j-blocks of the half
NTK = T // KB  # 16 tk blocks
GRP = 2  # tk-blocks per exp group (4 psum banks: 2 heads x 2 tk)
NGRP = NTK // GRP  # 8


def r(ap):
    return ap.bitcast(F32R)


def build_kernel():
    nc = bacc.Bacc(
        "TRN2", target_bir_lowering=False, debug=False, num_devices=N_CORES
    )
    xqT = nc.dram_tensor("xqT", [D, T], F32R, kind="ExternalInput").ap()
    xkT = nc.dram_tensor("xkT", [D, T], F32R, kind="ExternalInput").ap()
    xvT = nc.dram_tensor("xvT", [D, T], F32R, kind="ExternalInput").ap()
    wqT = nc.dram_tensor("wqT", [D, HALF], F32R, kind="ExternalInput").ap()
    wkT = nc.dram_tensor("wkT", [D, HALF], F32R, kind="ExternalInput").ap()
    wvT = nc.dram_tensor("wvT", [D, HALF], F32R, kind="ExternalInput").ap()
    woT = nc.dram_tensor("woT", [HALF, D], F32R, kind="ExternalInput").ap()
    bq = nc.dram_tensor("bq", [HALF, 1], F32, kind="ExternalInput").ap()
    bk = nc.dram_tensor("bk", [HALF, 1], F32, kind="ExternalInput").ap()
    ones_in = nc.dram_tensor("ones_in", [KB, H // 2], F32R, kind="ExternalInput").ap()
    partial = nc.dram_tensor("partial", [T, D], F32, kind="ExternalOutput").ap()

    with tile.TileContext(nc) as tc, ExitStack() as ctx:
        p_const = ctx.enter_context(tc.tile_pool(name="const", bufs=1))
        p_kt = ctx.enter_context(tc.tile_pool(name="kt", bufs=NJB))
        p_v = ctx.enter_context(tc.tile_pool(name="v", bufs=NTK))
        p_qt = ctx.enter_context(tc.tile_pool(name="qt", bufs=2 * NJB))
        p_xs = ctx.enter_context(tc.tile_pool(name="xs", bufs=3))
        p_ex = ctx.enter_context(tc.tile_pool(name="ex", bufs=2))
        p_ot = ctx.enter_context(tc.tile_pool(name="ot", bufs=2 * NJB))
        p_rc = ctx.enter_context(tc.tile_pool(name="rc", bufs=2))
        p_st = ctx.enter_context(tc.tile_pool(name="st", bufs=2))
        # PSUM: scores 4 banks + av 2 + proj/outproj 2 = 8
        p_sc = ctx.enter_context(tc.tile_pool(name="sc", bufs=1, space="PSUM"))
        p_av = ctx.enter_context(tc.tile_pool(name="av", bufs=2, space="PSUM"))
        p_po = ctx.enter_context(tc.tile_pool(name="po", bufs=2, space="PSUM"))

        # ---- constants ----
        w_q = p_const.tile([KB, NKB, HALF], F32R, tag="wq")
        nc.sync.dma_start(w_q[:], wqT.rearrange("(kb p) j -> p kb j", p=KB))
        w_k = p_const.tile([KB, NKB, HALF], F32R, tag="wk")
        nc.sync.dma_start(w_k[:], wkT.rearrange("(kb p) j -> p kb j", p=KB))
        w_v = p_const.tile([KB, NKB, HALF], F32R, tag="wv")
        nc.sync.dma_start(w_v[:], wvT.rearrange("(kb p) j -> p kb j", p=KB))
        w_o = p_const.tile([KB, NJB, D], F32R, tag="wo")
        nc.sync.dma_start(w_o[:], woT.rearrange("(jb p) n -> p jb n", p=KB))
        b_q = p_const.tile([KB, NJB], F32, tag="bq")
        nc.sync.dma_start(b_q[:], bq.rearrange("(jb p) one -> p (jb one)", p=KB))
        b_k = p_const.tile([KB, NJB], F32, tag="bk")
        nc.sync.dma_start(b_k[:], bk.rearrange("(jb p) one -> p (jb one)", p=KB))
        ones8 = p_const.tile([KB, H // 2], F32R, tag="ones8")
        nc.sync.dma_start(ones8[:], ones_in[:])

        # ---- K^T projection: KT[jb] is [128 (j), T] ----
        kt_tiles = [p_kt.tile([KB, T], F32R, tag="kt", name=f"kt{j}") for j in range(NJB)]
        for tb in range(NTB):
            ps = p_sc.tile([KB, 4 * TB], F32, tag="sc")
            for kb in range(NKB):
                xt = p_xs.tile([KB, TB], F32R, tag="xs")
                nc.sync.dma_start(xt[:], xkT[kb * KB : (kb + 1) * KB, tb * TB : (tb + 1) * TB])
                for jb in range(NJB):
                    nc.tensor.matmul(
                        ps[:, jb * TB : (jb + 1) * TB],
                        r(w_k[:, kb, jb * KB : (jb + 1) * KB]),
                        r(xt[:]),
                        start=(kb == 0),
                        stop=(kb == NKB - 1),
                    )
            for jb in range(NJB):
                nc.vector.tensor_scalar_add(
                    kt_tiles[jb][:, tb * TB : (tb + 1) * TB],
                    ps[:, jb * TB : (jb + 1) * TB],
                    b_k[:, jb : jb + 1],
                )

        # ---- V projection (natural layout): V[tk] is [128 (t), HALF (j)] ----
        v_tiles = [
            p_v.tile([KB, H // 2, DH + 1], F32R, tag="v", name=f"v{j}")
            for j in range(NTK)
        ]
        for t in range(NTK):
            nc.sync.dma_start(v_tiles[t][:, :, DH : DH + 1], ones8[:, :, None])
        for tb in range(NTB):
            ps = p_sc.tile([KB, 4 * TB], F32, tag="sc")
            for kb in range(NKB):
                xt = p_xs.tile([KB, TB], F32R, tag="xs")
                nc.sync.dma_start(xt[:], xvT[kb * KB : (kb + 1) * KB, tb * TB : (tb + 1) * TB])
                for ts in range(4):
                    nc.tensor.matmul(
                        ps[:, ts * TB : (ts + 1) * TB],
                        r(xt[:, ts * KB : (ts + 1) * KB]),
                        r(w_v[:, kb, :]),
                        start=(kb == 0),
                        stop=(kb == NKB - 1),
                    )
            for ts in range(4):
                nc.vector.tensor_copy(
                    v_tiles[tb * 4 + ts][:, :, 0:DH],
                    ps[:, ts * TB : (ts + 1) * TB].rearrange("p (h d) -> p h d", d=DH),
                )

        # ---- per t-block: Q^T projection, attention, out-projection ----
        for tq in range(NTB):
            # Q^T for this t-block: qt[jb] [128 (j), TB]
            qt_tiles = [p_qt.tile([KB, TB], F32R, tag="qt", name=f"qt{j}") for j in range(NJB)]
            ps = p_sc.tile([KB, 4 * TB], F32, tag="sc")
            for kb in range(NKB):
                xt = p_xs.tile([KB, TB], F32R, tag="xs")
                nc.sync.dma_start(xt[:], xqT[kb * KB : (kb + 1) * KB, tq * TB : (tq + 1) * TB])
                for jb in range(NJB):
                    nc.tensor.matmul(
                        ps[:, jb * TB : (jb + 1) * TB],
                        r(w_q[:, kb, jb * KB : (jb + 1) * KB]),
                        r(xt[:]),
                        start=(kb == 0),
                        stop=(kb == NKB - 1),
                    )
            for jb in range(NJB):
                nc.vector.tensor_scalar_add(
                    qt_tiles[jb][:], ps[:, jb * TB : (jb + 1) * TB], b_q[:, jb : jb + 1]
                )

            ot_tiles = [p_ot.tile([KB, TB], F32R, tag="ot", name=f"ot{j}") for j in range(NJB)]
            for jp in range(NJB):  # head pair (2*jp, 2*jp+1)
                # fp32r matmuls require dst partition base 0, so each head of
                # the pair accumulates into its own psum tile; row DH carries the
                # softmax denominator via the ones column interleaved in V.
                avs = [p_av.tile([DH + 1, TB], F32, tag="av", name=f"av{i}") for i in range(2)]
                for g in range(NGRP):
                    sc = p_sc.tile([KB, 4 * TB], F32, tag="sc")
                    # scores: S^T[tk-block, tq] for both heads of the pair
                    for i in range(2):
                        for u in range(GRP):
                            tk = g * GRP + u
                            slot = i * GRP + u
                            nc.tensor.matmul(
                                sc[:, slot * TB : (slot + 1) * TB],
                                r(kt_tiles[jp][i * DH : (i + 1) * DH, tk * KB : (tk + 1) * KB]),
                                r(qt_tiles[jp][i * DH : (i + 1) * DH, :]),
                                start=True,
                                stop=True,
                            )
                    ex = p_ex.tile([KB, 4 * TB], F32R, tag="ex")
                    nc.scalar.activation(
                        ex[:], sc[:], mybir.ActivationFunctionType.Exp, scale=0.125
                    )
                    for i in range(2):
                        for u in range(GRP):
                            tk = g * GRP + u
                            slot = i * GRP + u
                            exs = ex[:, slot * TB : (slot + 1) * TB]
                            nc.tensor.matmul(
                                avs[i][:],
                                r(v_tiles[tk][:, 2 * jp + i, :]),
                                r(exs),
                                start=(tk == 0),
                                stop=(tk == NTK - 1),
                            )
                for i in range(2):
                    # denominator row -> sbuf, DMA-broadcast across the 64
                    # head-dim partitions, reciprocal at base 0 (custom-DVE
                    # ops misbehave on nonzero partition base), then scale
                    dsb = p_rc.tile([DH + 1, TB], F32, tag="dsb")
                    nc.vector.tensor_copy(dsb[DH : DH + 1, :], avs[i][DH : DH + 1, :])
                    bc = p_rc.tile([DH, TB], F32, tag="bc")
                    nc.sync.dma_start(
                        bc[:],
                        dsb[DH : DH + 1, None, :].broadcast_to([1, DH, TB]),
                    )
                    rc2 = p_rc.tile([DH, TB], F32, tag="rc2")
                    nc.vector.reciprocal_approx_fast(rc2[:], bc[:])
                    if i == 0:
                        nc.vector.tensor_mul(ot_tiles[jp][0:DH, :], avs[i][0:DH, :], rc2[:])
                    else:
                        # DVE can't shift partitions; stage then DMA into rows 64:128
                        stg = p_rc.tile([DH, TB], F32R, tag="stg")
                        nc.vector.tensor_mul(stg[:], avs[i][0:DH, :], rc2[:])
                        nc.sync.dma_start(ot_tiles[jp][DH : 2 * DH, :], stg[:])

            # out-projection for this t-block
            for nb in range(2):
                for ts in range(4):
                    po = p_po.tile([KB, TB], F32, tag="po")
                    for jp in range(NJB):
                        nc.tensor.matmul(
                            po[:],
                            r(ot_tiles[jp][:, ts * KB : (ts + 1) * KB]),
                            r(w_o[:, jp, nb * TB : (nb + 1) * TB]),
                            start=(jp == 0),
                            stop=(jp == NJB - 1),
                        )
                    st = p_st.tile([KB, TB], F32, tag="st")
                    nc.vector.tensor_copy(st[:], po[:])
                    nc.sync.dma_start(
                        partial[
                            tq * TB + ts * KB : tq * TB + (ts + 1) * KB,
                            nb * TB : (nb + 1) * TB,
                        ],
                        st[:],
                    )

    nc.compile()
    return nc


def kernel(**inputs: np.ndarray) -> np.ndarray:
    query = np.asarray(inputs["query"], dtype=np.float32)
    key = np.asarray(inputs["key"], dtype=np.float32)
    value = np.asarray(inputs["value"], dtype=np.float32)
    w_q = np.asarray(inputs["w_q"], dtype=np.float32)
    b_q = np.asarray(inputs["b_q"], dtype=np.float32)
    w_k = np.asarray(inputs["w_k"], dtype=np.float32)
    b_k = np.asarray(inputs["b_k"], dtype=np.float32)
    w_v = np.asarray(inputs["w_v"], dtype=np.float32)
    b_v = np.asarray(inputs["b_v"], dtype=np.float32)
    w_o = np.asarray(inputs["w_o"], dtype=np.float32)
    b_o = np.asarray(inputs["b_o"], dtype=np.float32)

    nc = build_kernel()

    in_maps = []
    for c in range(N_CORES):
        b = c // 2
        hh = c % 2
        sl = slice(hh * HALF, (hh + 1) * HALF)
        in_maps.append(
            {
                "xqT": np.ascontiguousarray(query[b].T),
                "xkT": np.ascontiguousarray(key[b].T),
                "xvT": np.ascontiguousarray(value[b].T),
                "wqT": np.ascontiguousarray(w_q[sl, :].T),
                "wkT": np.ascontiguousarray(w_k[sl, :].T),
                "wvT": np.ascontiguousarray(w_v[sl, :].T),
                "woT": np.ascontiguousarray(w_o[:, sl].T),
                "bq": np.ascontiguousarray(b_q[sl].reshape(HALF, 1)),
                "bk": np.ascontiguousarray(b_k[sl].reshape(HALF, 1)),
                "ones_in": np.ones((KB, H // 2), dtype=np.float32),
            }
        )

    res = run_bass_kernel_spmd(nc, in_maps, core_ids=list(range(N_CORES)))

    const_row = (b_v[None, :] @ w_o.T + b_o[None, :]).astype(np.float32)
    out = np.empty((B, T, D), dtype=np.float32)
    for b in range(B):
        out[b] = res.results[2 * b]["partial"] + res.results[2 * b + 1]["partial"]
        out[b] += const_row
    return out



# revision 7
# speedup vs baseline: 2.0183x; 2.0183x over previous
"""Multi-head attention (B=4, T=2048, D=1024, H=16) on 8 TRN2 NeuronCores.

Sharding: core c handles batch b = c//2 and head-half hh = c%2 (8 heads,
512 of the 1024 channel dims). Each core computes its half of the head
outputs and a row-sharded output projection, producing a partial
[T, D] output. Host unshard: out[b] = partial[2b] + partial[2b+1]
+ b_o + b_v @ w_o.T (the value-bias contribution commutes through
attention because softmax rows sum to 1).

All matmuls run in bf16 (rel tolerance is 2e-2; bf16 with f32 psum
accumulation lands ~5e-3). bf16 halves weight-load cost on the PE and
every matmul streams 1024 moving rows per LDWEIGHTS (compound matmul
across two psum banks), which removes the fp32r weight-load overhead
that dominated the fp32r version. The softmax exp runs on ACT in
[128, 1024] tiles double-buffered in PSUM so PE and ACT pipeline.
"""

from contextlib import ExitStack

import numpy as np

import concourse.bass as bass
import concourse.mybir as mybir
import concourse.tile as tile
from concourse import bacc
from concourse.bass_utils import run_bass_kernel_spmd

B, T, D = 4, 2048, 1024
H = 16
DH = 64  # head dim
HALF = 512  # channels per core (8 heads)
N_CORES = 8

F32 = mybir.dt.float32
BF16 = mybir.dt.bfloat16

KB = 128  # contraction / partition block
NKB = D // KB  # 8
NJB = HALF // KB  # 4 j-blocks of the half
NTK = T // KB  # 16 key blocks
QH = 1024  # query half streamed per scores matmul
NQH = T // QH  # 2


def build_kernel():
    nc = bacc.Bacc(
        "TRN2", target_bir_lowering=False, debug=False, num_devices=N_CORES
    )
    xqT = nc.dram_tensor("xqT", [D, T], BF16, kind="ExternalInput").ap()
    xkT = nc.dram_tensor("xkT", [D, T], BF16, kind="ExternalInput").ap()
    xvT = nc.dram_tensor("xvT", [D, T], BF16, kind="ExternalInput").ap()
    wqT = nc.dram_tensor("wqT", [D, HALF], BF16, kind="ExternalInput").ap()
    wkT = nc.dram_tensor("wkT", [D, HALF], BF16, kind="ExternalInput").ap()
    wvT = nc.dram_tensor("wvT", [D, HALF], BF16, kind="ExternalInput").ap()
    woT = nc.dram_tensor("woT", [HALF, D], BF16, kind="ExternalInput").ap()
    bq = nc.dram_tensor("bq", [HALF, 1], F32, kind="ExternalInput").ap()
    bk = nc.dram_tensor("bk", [HALF, 1], F32, kind="ExternalInput").ap()
    ones_in = nc.dram_tensor("ones_in", [KB, H // 2], BF16, kind="ExternalInput").ap()
    partial = nc.dram_tensor("partial", [T, D], F32, kind="ExternalOutput").ap()

    with tile.TileContext(nc) as tc, ExitStack() as ctx:
        p_const = ctx.enter_context(tc.tile_pool(name="const", bufs=1))
        p_x = ctx.enter_context(tc.tile_pool(name="x", bufs=2))
        p_kt = ctx.enter_context(tc.tile_pool(name="kt", bufs=NJB))
        p_qt = ctx.enter_context(tc.tile_pool(name="qt", bufs=NJB))
        p_ot = ctx.enter_context(tc.tile_pool(name="ot", bufs=NJB))
        p_v = ctx.enter_context(tc.tile_pool(name="v", bufs=NTK))
        p_ex = ctx.enter_context(tc.tile_pool(name="ex", bufs=3))
        p_dr = ctx.enter_context(tc.tile_pool(name="dr", bufs=2))
        p_st = ctx.enter_context(tc.tile_pool(name="st", bufs=2))
        # PSUM: pool A = 2 bufs x 2 banks (proj / scores / out-proj),
        #       pool B = 2 bufs x 2 banks (AV accumulators)
        p_A = ctx.enter_context(tc.tile_pool(name="A", bufs=2, space="PSUM"))
        p_B = ctx.enter_context(tc.tile_pool(name="B", bufs=2, space="PSUM"))

        # ---- constants ----
        w_q = p_const.tile([KB, NKB, HALF], BF16, tag="wq")
        nc.sync.dma_start(w_q[:], wqT.rearrange("(kb p) j -> p kb j", p=KB))
        w_k = p_const.tile([KB, NKB, HALF], BF16, tag="wk")
        nc.sync.dma_start(w_k[:], wkT.rearrange("(kb p) j -> p kb j", p=KB))
        w_v = p_const.tile([KB, NKB, HALF], BF16, tag="wv")
        nc.sync.dma_start(w_v[:], wvT.rearrange("(kb p) j -> p kb j", p=KB))
        w_o = p_const.tile([KB, NJB, D], BF16, tag="wo")
        nc.sync.dma_start(w_o[:], woT.rearrange("(jb p) n -> p jb n", p=KB))
        b_q = p_const.tile([KB, NJB], F32, tag="bq")
        nc.sync.dma_start(b_q[:], bq.rearrange("(jb p) one -> p (jb one)", p=KB))
        b_k = p_const.tile([KB, NJB], F32, tag="bk")
        nc.sync.dma_start(b_k[:], bk.rearrange("(jb p) one -> p (jb one)", p=KB))
        ones8 = p_const.tile([KB, H // 2], BF16, tag="ones8")
        nc.sync.dma_start(ones8[:], ones_in[:])

        # ---- persistent sbuf tiles ----
        kt_tiles = [p_kt.tile([KB, T], BF16, tag="kt", name=f"kt{j}") for j in range(NJB)]
        qt_tiles = [p_qt.tile([KB, T], BF16, tag="qt", name=f"qt{j}") for j in range(NJB)]
        ot_tiles = [p_ot.tile([KB, T], BF16, tag="ot", name=f"ot{j}") for j in range(NJB)]
        v_tiles = [
            p_v.tile([KB, H // 2, DH + 1], BF16, tag="v", name=f"v{t}")
            for t in range(NTK)
        ]
        for t in range(NTK):
            nc.sync.dma_start(v_tiles[t][:, :, DH : DH + 1], ones8[:, :, None])

        # ---- stage inputs (x pool ring: xk -> buf0, xv -> buf1, xq -> buf0) ----
        xk = p_x.tile([KB, NKB, T], BF16, tag="x", name="xk")
        for kb in range(NKB):
            nc.sync.dma_start(xk[:, kb, :], xkT[kb * KB : (kb + 1) * KB, :])
        xv = p_x.tile([KB, NKB, T], BF16, tag="x", name="xv")
        for kb in range(NKB):
            nc.sync.dma_start(xv[:, kb, :], xvT[kb * KB : (kb + 1) * KB, :])

        # ---- K^T projection: kt[jb] = [128 j, T] ----
        # matmul outputs may not cross a PSUM bank -> 512-col pieces
        for jb in range(NJB):
            for th in range(NQH):
                ps = p_A.tile([KB, QH], F32, tag="mm")
                for kb in range(NKB):
                    for s in range(2):
                        nc.tensor.matmul(
                            ps[:, s * 512 : (s + 1) * 512],
                            w_k[:, kb, jb * KB : (jb + 1) * KB],
                            xk[:, kb, th * QH + s * 512 : th * QH + (s + 1) * 512],
                            start=(kb == 0),
                            stop=(kb == NKB - 1),
                        )
                nc.vector.tensor_scalar_add(
                    kt_tiles[jb][:, th * QH : (th + 1) * QH], ps[:], b_k[:, jb : jb + 1]
                )

        # xq reuses buf0 after the K-proj matmuls release xk
        xq = p_x.tile([KB, NKB, T], BF16, tag="x", name="xq")
        for kb in range(NKB):
            nc.sync.dma_start(xq[:, kb, :], xqT[kb * KB : (kb + 1) * KB, :])

        # ---- V projection (natural layout): v[t] = [128 t, 8 h, 65] ----
        for tb in range(NTK):
            ps = p_A.tile([KB, HALF], F32, tag="mm")
            for kb in range(NKB):
                nc.tensor.matmul(
                    ps[:],
                    xv[:, kb, tb * KB : (tb + 1) * KB],
                    w_v[:, kb, :],
                    start=(kb == 0),
                    stop=(kb == NKB - 1),
                )
            nc.vector.tensor_copy(
                v_tiles[tb][:, :, 0:DH], ps.rearrange("p (h d) -> p h d", d=DH)
            )

        # ---- Q^T projection: qt[jb] = [128 j, T] ----
        for jb in range(NJB):
            for th in range(NQH):
                ps = p_A.tile([KB, QH], F32, tag="mm")
                for kb in range(NKB):
                    for s in range(2):
                        nc.tensor.matmul(
                            ps[:, s * 512 : (s + 1) * 512],
                            w_q[:, kb, jb * KB : (jb + 1) * KB],
                            xq[:, kb, th * QH + s * 512 : th * QH + (s + 1) * 512],
                            start=(kb == 0),
                            stop=(kb == NKB - 1),
                        )
                nc.vector.tensor_scalar_add(
                    qt_tiles[jb][:, th * QH : (th + 1) * QH], ps[:], b_q[:, jb : jb + 1]
                )

        # ---- attention + interleaved out-projection ----
        def emit_scores(h, qh, tk):
            jp, hi = h // 2, h % 2
            sc = p_A.tile([KB, QH], F32, tag="mm", name="sc")
            for s in range(2):
                nc.tensor.matmul(
                    sc[:, s * 512 : (s + 1) * 512],
                    kt_tiles[jp][hi * DH : (hi + 1) * DH, tk * KB : (tk + 1) * KB],
                    qt_tiles[jp][
                        hi * DH : (hi + 1) * DH,
                        qh * QH + s * 512 : qh * QH + (s + 1) * 512,
                    ],
                    start=True,
                    stop=True,
                )
            ex = p_ex.tile([KB, QH], BF16, tag="ex")
            nc.scalar.activation(
                ex[:], sc[:], mybir.ActivationFunctionType.Exp, scale=0.125
            )
            return ex

        def emit_av(h, qh, tk, ex, av):
            for s in range(2):
                nc.tensor.matmul(
                    av[:, s * 512 : (s + 1) * 512],
                    v_tiles[tk][:, h, :],
                    ex[:, s * 512 : (s + 1) * 512],
                    start=(tk == 0),
                    stop=(tk == NTK - 1),
                )

        def emit_drain(h, qh, av):
            jp, hi = h // 2, h % 2
            dsb = p_dr.tile([DH + 1, QH], F32, tag="dsb")
            nc.vector.tensor_copy(dsb[DH : DH + 1, :], av[DH : DH + 1, :])
            bc = p_dr.tile([DH, QH], F32, tag="bc")
            nc.sync.dma_start(
                bc[:], dsb[DH : DH + 1, None, :].broadcast_to([1, DH, QH])
            )
            rc = p_dr.tile([DH, QH], F32, tag="rc")
            nc.vector.reciprocal_approx_fast(rc[:], bc[:])
            dst = ot_tiles[jp][hi * DH : (hi + 1) * DH, qh * QH : (qh + 1) * QH]
            if hi == 0:
                nc.vector.tensor_mul(dst, av[0:DH, :], rc[:])
            else:
                stg = p_dr.tile([DH, QH], BF16, tag="stg")
                nc.vector.tensor_mul(stg[:], av[0:DH, :], rc[:])
                nc.sync.dma_start(dst, stg[:])

        def emit_outproj(tblk):
            po = p_A.tile([KB, D], F32, tag="mm", name="po")
            for jp in range(NJB):
                for s in range(2):
                    nc.tensor.matmul(
                        po[:, s * 512 : (s + 1) * 512],
                        ot_tiles[jp][:, tblk * KB : (tblk + 1) * KB],
                        w_o[:, jp, s * 512 : (s + 1) * 512],
                        start=(jp == 0),
                        stop=(jp == NJB - 1),
                    )
            st = p_st.tile([KB, D], F32, tag="st")
            nc.vector.tensor_copy(st[:], po[:])
            nc.sync.dma_start(partial[tblk * KB : (tblk + 1) * KB, :], st[:])

        pending_av = None  # (h, qh, tk, ex, av)
        for qh in range(NQH):
            for h in range(H // 2):
                av = p_B.tile([DH + 1, QH], F32, tag="av", name="av")
                for tk in range(NTK):
                    ex = emit_scores(h, qh, tk)
                    if pending_av is not None:
                        ph, pqh, ptk, pex, pav = pending_av
                        emit_av(ph, pqh, ptk, pex, pav)
                        if ptk == NTK - 1:
                            emit_drain(ph, pqh, pav)
                    pending_av = (h, qh, tk, ex, av)
                # during the second query half, all heads are done with
                # query-half 0 -> stream the first 8 out-proj blocks
                if qh == 1:
                    emit_outproj(h)
        ph, pqh, ptk, pex, pav = pending_av
        emit_av(ph, pqh, ptk, pex, pav)
        emit_drain(ph, pqh, pav)
        for tblk in range(8, NTK):
            emit_outproj(tblk)

    nc.compile()
    return nc


def kernel(**inputs: np.ndarray) -> np.ndarray:
    import ml_dtypes

    BF = ml_dtypes.bfloat16

    query = np.asarray(inputs["query"], dtype=np.float32)
    key = np.asarray(inputs["key"], dtype=np.float32)
    value = np.asarray(inputs["value"], dtype=np.float32)
    w_q = np.asarray(inputs["w_q"], dtype=np.float32)
    b_q = np.asarray(inputs["b_q"], dtype=np.float32)
    w_k = np.asarray(inputs["w_k"], dtype=np.float32)
    b_k = np.asarray(inputs["b_k"], dtype=np.float32)
    w_v = np.asarray(inputs["w_v"], dtype=np.float32)
    b_v = np.asarray(inputs["b_v"], dtype=np.float32)
    w_o = np.asarray(inputs["w_o"], dtype=np.float32)
    b_o = np.asarray(inputs["b_o"], dtype=np.float32)

    nc = build_kernel()

    in_maps = []
    for c in range(N_CORES):
        b = c // 2
        hh = c % 2
        sl = slice(hh * HALF, (hh + 1) * HALF)
        in_maps.append(
            {
                "xqT": np.ascontiguousarray(query[b].T).astype(BF),
                "xkT": np.ascontiguousarray(key[b].T).astype(BF),
                "xvT": np.ascontiguousarray(value[b].T).astype(BF),
                "wqT": np.ascontiguousarray(w_q[sl, :].T).astype(BF),
                "wkT": np.ascontiguousarray(w_k[sl, :].T).astype(BF),
                "wvT": np.ascontiguousarray(w_v[sl, :].T).astype(BF),
                "woT": np.ascontiguousarray(w_o[:, sl].T).astype(BF),
                "bq": np.ascontiguousarray(b_q[sl].reshape(HALF, 1)),
                "bk": np.ascontiguousarray(b_k[sl].reshape(HALF, 1)),
                "ones_in": np.ones((KB, H // 2), dtype=BF),
            }
        )

    res = run_bass_kernel_spmd(nc, in_maps, core_ids=list(range(N_CORES)))

    const_row = (b_v[None, :] @ w_o.T + b_o[None, :]).astype(np.float32)
    out = np.empty((B, T, D), dtype=np.float32)
    for b in range(B):
        out[b] = np.asarray(res.results[2 * b]["partial"], dtype=np.float32)
        out[b] += np.asarray(res.results[2 * b + 1]["partial"], dtype=np.float32)
        out[b] += const_row
    return out


# revision 8
# speedup vs baseline: 2.0277x; 1.0047x over previous
"""Multi-head attention (B=4, T=2048, D=1024, H=16) on 8 TRN2 NeuronCores.

Sharding: core c handles batch b = c//2 and head-half hh = c%2 (8 heads,
512 of the 1024 channel dims). Each core computes its half of the head
outputs and a row-sharded output projection, producing a partial
[T, D] output. Host unshard: out[b] = partial[2b] + partial[2b+1]
+ b_o + b_v @ w_o.T (the value-bias contribution commutes through
attention because softmax rows sum to 1).

All matmuls in bf16 (tolerance 2e-2; bf16 with f32 psum accumulation
lands ~4e-3). Phase structure minimizes the serial prologue: K-proj
streams kb-outer behind its own DMA, attention starts right after the
first query-half of Q-proj, and the second Q-proj half plus the output
projection are injected at attention head boundaries. Output-tile DMAs
ride the SWDGE (gpsimd) ring so the softmax-denominator drain DMAs on
the SP ring never queue behind them.
"""

from contextlib import ExitStack

import numpy as np

import concourse.bass as bass
import concourse.mybir as mybir
import concourse.tile as tile
from concourse import bacc
from concourse.bass_utils import run_bass_kernel_spmd

B, T, D = 4, 2048, 1024
H = 16
DH = 64  # head dim
HALF = 512  # channels per core (8 heads)
N_CORES = 8

F32 = mybir.dt.float32
BF16 = mybir.dt.bfloat16

KB = 128  # contraction / partition block
NKB = D // KB  # 8
NJB = HALF // KB  # 4 j-blocks of the half
NTK = T // KB  # 16 key blocks
QH = 1024  # query half streamed per scores tile
NQH = T // QH  # 2


def build_kernel():
    nc = bacc.Bacc(
        "TRN2", target_bir_lowering=False, debug=False, num_devices=N_CORES
    )
    xqT = nc.dram_tensor("xqT", [D, T], BF16, kind="ExternalInput").ap()
    xkT = nc.dram_tensor("xkT", [D, T], BF16, kind="ExternalInput").ap()
    xvT = nc.dram_tensor("xvT", [D, T], BF16, kind="ExternalInput").ap()
    wqT = nc.dram_tensor("wqT", [D, HALF], BF16, kind="ExternalInput").ap()
    wkT = nc.dram_tensor("wkT", [D, HALF], BF16, kind="ExternalInput").ap()
    wvT = nc.dram_tensor("wvT", [D, HALF], BF16, kind="ExternalInput").ap()
    woT = nc.dram_tensor("woT", [HALF, D], BF16, kind="ExternalInput").ap()
    bq = nc.dram_tensor("bq", [HALF, 1], F32, kind="ExternalInput").ap()
    bk = nc.dram_tensor("bk", [HALF, 1], F32, kind="ExternalInput").ap()
    ones_in = nc.dram_tensor("ones_in", [KB, H // 2], BF16, kind="ExternalInput").ap()
    partial = nc.dram_tensor("partial", [T, D], BF16, kind="ExternalOutput").ap()

    with tile.TileContext(nc) as tc, ExitStack() as ctx:
        p_const = ctx.enter_context(tc.tile_pool(name="const", bufs=1))
        p_x = ctx.enter_context(tc.tile_pool(name="x", bufs=2))
        p_kt = ctx.enter_context(tc.tile_pool(name="kt", bufs=NJB))
        p_qt = ctx.enter_context(tc.tile_pool(name="qt", bufs=NJB))
        p_ot = ctx.enter_context(tc.tile_pool(name="ot", bufs=NJB))
        p_v = ctx.enter_context(tc.tile_pool(name="v", bufs=NTK))
        p_ex = ctx.enter_context(tc.tile_pool(name="ex", bufs=4))
        p_dr = ctx.enter_context(tc.tile_pool(name="dr", bufs=2))
        p_st = ctx.enter_context(tc.tile_pool(name="st", bufs=2))
        # PSUM: pool A = 2 bufs x 2 banks, pool B = 2 bufs x 2 banks.
        # Phase 1 K/Q-proj borrows both pools for 4 live accumulators;
        # attention uses A for scores / out-proj and B for AV.
        p_A = ctx.enter_context(tc.tile_pool(name="A", bufs=2, space="PSUM"))
        p_B = ctx.enter_context(tc.tile_pool(name="B", bufs=2, space="PSUM"))

        # ---- input DMAs, in consumption order (SP ring is FIFO) ----
        w_k = p_const.tile([KB, NKB, HALF], BF16, tag="wk")
        nc.sync.dma_start(w_k[:], wkT.rearrange("(kb p) j -> p kb j", p=KB))
        xk = p_x.tile([KB, NKB, T], BF16, tag="x", name="xk")
        for kb in range(NKB):
            nc.sync.dma_start(xk[:, kb, :], xkT[kb * KB : (kb + 1) * KB, :])
        w_v = p_const.tile([KB, NKB, HALF], BF16, tag="wv")
        nc.sync.dma_start(w_v[:], wvT.rearrange("(kb p) j -> p kb j", p=KB))
        xv = p_x.tile([KB, NKB, T], BF16, tag="x", name="xv")
        for kb in range(NKB):
            nc.sync.dma_start(xv[:, kb, :], xvT[kb * KB : (kb + 1) * KB, :])
        w_q = p_const.tile([KB, NKB, HALF], BF16, tag="wq")
        nc.sync.dma_start(w_q[:], wqT.rearrange("(kb p) j -> p kb j", p=KB))
        b_q = p_const.tile([KB, NJB], F32, tag="bq")
        nc.sync.dma_start(b_q[:], bq.rearrange("(jb p) one -> p (jb one)", p=KB))
        b_k = p_const.tile([KB, NJB], F32, tag="bk")
        nc.sync.dma_start(b_k[:], bk.rearrange("(jb p) one -> p (jb one)", p=KB))
        ones8 = p_const.tile([KB, H // 2], BF16, tag="ones8")
        nc.sync.dma_start(ones8[:], ones_in[:])
        w_o = p_const.tile([KB, NJB, D], BF16, tag="wo")
        nc.sync.dma_start(w_o[:], woT.rearrange("(jb p) n -> p jb n", p=KB))

        # ---- persistent sbuf tiles ----
        kt_tiles = [p_kt.tile([KB, T], BF16, tag="kt", name=f"kt{j}") for j in range(NJB)]
        qt_tiles = [p_qt.tile([KB, T], BF16, tag="qt", name=f"qt{j}") for j in range(NJB)]
        ot_tiles = [p_ot.tile([KB, T], BF16, tag="ot", name=f"ot{j}") for j in range(NJB)]
        v_tiles = [
            p_v.tile([KB, H // 2, DH + 1], BF16, tag="v", name=f"v{t}")
            for t in range(NTK)
        ]
        for t in range(NTK):
            nc.sync.dma_start(v_tiles[t][:, :, DH : DH + 1], ones8[:, :, None])

        # ---- K^T projection, kb-outer so it streams behind the xk DMA ----
        # 4 live accumulators [128, 1024] = all 8 psum banks (pools A+B)
        for th in range(NQH):
            ps = [
                (p_A if jb < 2 else p_B).tile(
                    [KB, QH], F32, tag=("mm" if jb < 2 else "av"), name=f"kp{jb}"
                )
                for jb in range(NJB)
            ]
            for kb in range(NKB):
                for jb in range(NJB):
                    for s in range(2):
                        nc.tensor.matmul(
                            ps[jb][:, s * 512 : (s + 1) * 512],
                            w_k[:, kb, jb * KB : (jb + 1) * KB],
                            xk[:, kb, th * QH + s * 512 : th * QH + (s + 1) * 512],
                            start=(kb == 0),
                            stop=(kb == NKB - 1),
                        )
            for jb in range(NJB):
                nc.vector.tensor_scalar_add(
                    kt_tiles[jb][:, th * QH : (th + 1) * QH],
                    ps[jb][:],
                    b_k[:, jb : jb + 1],
                )

        # xq reuses the xk buffer once K-proj is done with it
        xq = p_x.tile([KB, NKB, T], BF16, tag="x", name="xq")
        for kb in range(NKB):
            nc.sync.dma_start(xq[:, kb, :], xqT[kb * KB : (kb + 1) * KB, :])

        # ---- V projection (natural layout): v[t] = [128 t, 8 h, 65] ----
        for tb in range(NTK):
            ps = p_A.tile([KB, HALF], F32, tag="mm")
            for kb in range(NKB):
                nc.tensor.matmul(
                    ps[:],
                    xv[:, kb, tb * KB : (tb + 1) * KB],
                    w_v[:, kb, :],
                    start=(kb == 0),
                    stop=(kb == NKB - 1),
                )
            nc.vector.tensor_copy(
                v_tiles[tb][:, :, 0:DH], ps.rearrange("p (h d) -> p h d", d=DH)
            )

        # ---- Q^T projection for one query-half (kb-inner, jb groups) ----
        def emit_qproj(jb, th):
            ps = p_A.tile([KB, QH], F32, tag="mm", name="qp")
            for kb in range(NKB):
                for s in range(2):
                    nc.tensor.matmul(
                        ps[:, s * 512 : (s + 1) * 512],
                        w_q[:, kb, jb * KB : (jb + 1) * KB],
                        xq[:, kb, th * QH + s * 512 : th * QH + (s + 1) * 512],
                        start=(kb == 0),
                        stop=(kb == NKB - 1),
                    )
            nc.vector.tensor_scalar_add(
                qt_tiles[jb][:, th * QH : (th + 1) * QH], ps[:], b_q[:, jb : jb + 1]
            )

        for jb in range(NJB):
            emit_qproj(jb, 0)

        # ---- attention with interleaved Q-th1 / out-projection ----
        def emit_scores(h, qh, tk):
            jp, hi = h // 2, h % 2
            sc = p_A.tile([KB, QH], F32, tag="mm", name="sc")
            for s in range(2):
                nc.tensor.matmul(
                    sc[:, s * 512 : (s + 1) * 512],
                    kt_tiles[jp][hi * DH : (hi + 1) * DH, tk * KB : (tk + 1) * KB],
                    qt_tiles[jp][
                        hi * DH : (hi + 1) * DH,
                        qh * QH + s * 512 : qh * QH + (s + 1) * 512,
                    ],
                    start=True,
                    stop=True,
                )
            ex = p_ex.tile([KB, QH], BF16, tag="ex")
            nc.scalar.activation(
                ex[:], sc[:], mybir.ActivationFunctionType.Exp, scale=0.125
            )
            return ex

        def emit_av(h, qh, tk, ex, av):
            for s in range(2):
                nc.tensor.matmul(
                    av[:, s * 512 : (s + 1) * 512],
                    v_tiles[tk][:, h, :],
                    ex[:, s * 512 : (s + 1) * 512],
                    start=(tk == 0),
                    stop=(tk == NTK - 1),
                )

        def emit_drain(h, qh, av):
            jp, hi = h // 2, h % 2
            dsb = p_dr.tile([DH + 1, QH], F32, tag="dsb")
            nc.vector.tensor_copy(dsb[DH : DH + 1, :], av[DH : DH + 1, :])
            bc = p_dr.tile([DH, QH], F32, tag="bc")
            nc.sync.dma_start(
                bc[:], dsb[DH : DH + 1, None, :].broadcast_to([1, DH, QH])
            )
            rc = p_dr.tile([DH, QH], F32, tag="rc")
            nc.vector.reciprocal_approx_fast(rc[:], bc[:])
            dst = ot_tiles[jp][hi * DH : (hi + 1) * DH, qh * QH : (qh + 1) * QH]
            if hi == 0:
                nc.vector.tensor_mul(dst, av[0:DH, :], rc[:])
            else:
                stg = p_dr.tile([DH, QH], BF16, tag="stg")
                nc.vector.tensor_mul(stg[:], av[0:DH, :], rc[:])
                nc.sync.dma_start(dst, stg[:])

        def emit_outproj(tblk):
            po = p_A.tile([KB, D], F32, tag="mm", name="po")
            for jp in range(NJB):
                for s in range(2):
                    nc.tensor.matmul(
                        po[:, s * 512 : (s + 1) * 512],
                        ot_tiles[jp][:, tblk * KB : (tblk + 1) * KB],
                        w_o[:, jp, s * 512 : (s + 1) * 512],
                        start=(jp == 0),
                        stop=(jp == NJB - 1),
                    )
            st = p_st.tile([KB, D], BF16, tag="st")
            nc.vector.tensor_copy(st[:], po[:])
            # SWDGE ring: keep the SP ring free for the drain DMAs
            nc.gpsimd.dma_start(partial[tblk * KB : (tblk + 1) * KB, :], st[:])

        pending_av = None  # (h, qh, tk, ex, av)
        for qh in range(NQH):
            for h in range(H // 2):
                av = p_B.tile([DH + 1, QH], F32, tag="av", name="av")
                for tk in range(NTK):
                    ex = emit_scores(h, qh, tk)
                    if pending_av is not None:
                        ph, pqh, ptk, pex, pav = pending_av
                        emit_av(ph, pqh, ptk, pex, pav)
                        if ptk == NTK - 1:
                            emit_drain(ph, pqh, pav)
                    pending_av = (h, qh, tk, ex, av)
                # head boundary: inject deferred PE work
                if qh == 0 and h < NJB:
                    emit_qproj(h, 1)
                if qh == 1:
                    emit_outproj(h)
        ph, pqh, ptk, pex, pav = pending_av
        emit_av(ph, pqh, ptk, pex, pav)
        emit_drain(ph, pqh, pav)
        for tblk in range(8, NTK):
            emit_outproj(tblk)

    nc.compile()
    return nc


def kernel(**inputs: np.ndarray) -> np.ndarray:
    import ml_dtypes

    BF = ml_dtypes.bfloat16

    query = np.asarray(inputs["query"], dtype=np.float32)
    key = np.asarray(inputs["key"], dtype=np.float32)
    value = np.asarray(inputs["value"], dtype=np.float32)
    w_q = np.asarray(inputs["w_q"], dtype=np.float32)
    b_q = np.asarray(inputs["b_q"], dtype=np.float32)
    w_k = np.asarray(inputs["w_k"], dtype=np.float32)
    b_k = np.asarray(inputs["b_k"], dtype=np.float32)
    w_v = np.asarray(inputs["w_v"], dtype=np.float32)
    b_v = np.asarray(inputs["b_v"], dtype=np.float32)
    w_o = np.asarray(inputs["w_o"], dtype=np.float32)
    b_o = np.asarray(inputs["b_o"], dtype=np.float32)

    nc = build_kernel()

    in_maps = []
    for c in range(N_CORES):
        b = c // 2
        hh = c % 2
        sl = slice(hh * HALF, (hh + 1) * HALF)
        in_maps.append(
            {
                "xqT": np.ascontiguousarray(query[b].T).astype(BF),
                "xkT": np.ascontiguousarray(key[b].T).astype(BF),
                "xvT": np.ascontiguousarray(value[b].T).astype(BF),
                "wqT": np.ascontiguousarray(w_q[sl, :].T).astype(BF),
                "wkT": np.ascontiguousarray(w_k[sl, :].T).astype(BF),
                "wvT": np.ascontiguousarray(w_v[sl, :].T).astype(BF),
                "woT": np.ascontiguousarray(w_o[:, sl].T).astype(BF),
                "bq": np.ascontiguousarray(b_q[sl].reshape(HALF, 1)),
                "bk": np.ascontiguousarray(b_k[sl].reshape(HALF, 1)),
                "ones_in": np.ones((KB, H // 2), dtype=BF),
            }
        )

    res = run_bass_kernel_spmd(nc, in_maps, core_ids=list(range(N_CORES)))

    const_row = (b_v[None, :] @ w_o.T + b_o[None, :]).astype(np.float32)
    out = np.empty((B, T, D), dtype=np.float32)
    for b in range(B):
        out[b] = np.asarray(res.results[2 * b]["partial"], dtype=np.float32)
        out[b] += np.asarray(res.results[2 * b + 1]["partial"], dtype=np.float32)
        out[b] += const_row
    return out


# revision 13
# speedup vs baseline: 2.1436x; 1.0572x over previous
"""Multi-head attention (B=4, T=2048, D=1024, H=16) on 8 TRN2 NeuronCores.

Sharding: core c handles batch b = c//2 and head-half hh = c%2 (8 heads,
512 of the 1024 channel dims). Each core computes its half of the head
outputs and a row-sharded output projection, producing a partial
[T, D] output. Host unshard: out[b] = partial[2b] + partial[2b+1]
+ b_o + b_v @ w_o.T (the value-bias contribution commutes through
attention because softmax rows sum to 1).

All matmuls in bf16 (tolerance 2e-2; bf16 with f32 psum accumulation
lands ~4e-3). Phase structure minimizes the serial prologue: K-proj
streams kb-outer behind its own DMA, attention starts right after the
first query-half of Q-proj, and the second Q-proj half plus the output
projection are injected at attention head boundaries. Output-tile DMAs
ride the SWDGE (gpsimd) ring so the softmax-denominator drain DMAs on
the SP ring never queue behind them.
"""

from contextlib import ExitStack

import numpy as np

import concourse.bass as bass
import concourse.mybir as mybir
import concourse.tile as tile
from concourse import bacc
from concourse.bass_utils import run_bass_kernel_spmd

B, T, D = 4, 2048, 1024
H = 16
DH = 64  # head dim
HALF = 512  # channels per core (8 heads)
N_CORES = 8

F32 = mybir.dt.float32
BF16 = mybir.dt.bfloat16

KB = 128  # contraction / partition block
NKB = D // KB  # 8
NJB = HALF // KB  # 4 j-blocks of the half
NTK = T // KB  # 16 key blocks
QH = 1024  # query half streamed per scores tile
NQH = T // QH  # 2


def build_kernel():
    nc = bacc.Bacc(
        "TRN2", target_bir_lowering=False, debug=False, num_devices=N_CORES
    )
    xqT = nc.dram_tensor("xqT", [D, T], BF16, kind="ExternalInput").ap()
    xkT = nc.dram_tensor("xkT", [D, T], BF16, kind="ExternalInput").ap()
    xvT = nc.dram_tensor("xvT", [D, T], BF16, kind="ExternalInput").ap()
    wqT = nc.dram_tensor("wqT", [D, HALF], BF16, kind="ExternalInput").ap()
    wkT = nc.dram_tensor("wkT", [D, HALF], BF16, kind="ExternalInput").ap()
    wvT = nc.dram_tensor("wvT", [D, HALF], BF16, kind="ExternalInput").ap()
    woT = nc.dram_tensor("woT", [HALF, D], BF16, kind="ExternalInput").ap()
    bq = nc.dram_tensor("bq", [HALF, 1], F32, kind="ExternalInput").ap()
    bk = nc.dram_tensor("bk", [HALF, 1], F32, kind="ExternalInput").ap()
    ones_in = nc.dram_tensor("ones_in", [KB, H // 2], BF16, kind="ExternalInput").ap()
    partial = nc.dram_tensor("partial", [T, D], BF16, kind="ExternalOutput").ap()

    with tile.TileContext(nc) as tc, ExitStack() as ctx:
        p_const = ctx.enter_context(tc.tile_pool(name="const", bufs=1))
        p_x = ctx.enter_context(tc.tile_pool(name="x", bufs=2))
        p_kt = ctx.enter_context(tc.tile_pool(name="kt", bufs=NJB))
        p_qt = ctx.enter_context(tc.tile_pool(name="qt", bufs=NJB))
        p_ot = ctx.enter_context(tc.tile_pool(name="ot", bufs=NJB))
        p_v = ctx.enter_context(tc.tile_pool(name="v", bufs=NTK))
        p_ex = ctx.enter_context(tc.tile_pool(name="ex", bufs=4))
        p_dr = ctx.enter_context(tc.tile_pool(name="dr", bufs=2))
        p_st = ctx.enter_context(tc.tile_pool(name="st", bufs=2))
        # PSUM: pool A = 2 bufs x 2 banks, pool B = 2 bufs x 2 banks.
        # Phase 1 K/Q-proj borrows both pools for 4 live accumulators;
        # attention uses A for scores / out-proj and B for AV.
        p_A = ctx.enter_context(tc.tile_pool(name="A", bufs=2, space="PSUM"))
        p_B = ctx.enter_context(tc.tile_pool(name="B", bufs=2, space="PSUM"))

        # ---- input DMAs, in consumption order (SP ring is FIFO) ----
        b_q = p_const.tile([KB, NJB], F32, tag="bq")
        nc.sync.dma_start(b_q[:], bq.rearrange("(jb p) one -> p (jb one)", p=KB))
        b_k = p_const.tile([KB, NJB], F32, tag="bk")
        nc.sync.dma_start(b_k[:], bk.rearrange("(jb p) one -> p (jb one)", p=KB))
        ones8 = p_const.tile([KB, H // 2], BF16, tag="ones8")
        nc.sync.dma_start(ones8[:], ones_in[:])
        w_k = p_const.tile([KB, NKB, HALF], BF16, tag="wk")
        nc.sync.dma_start(w_k[:], wkT.rearrange("(kb p) j -> p kb j", p=KB))
        xk = p_x.tile([KB, NKB, T], BF16, tag="x", name="xk")
        for kb in range(NKB):
            nc.sync.dma_start(xk[:, kb, :], xkT[kb * KB : (kb + 1) * KB, :])
        w_q = p_const.tile([KB, NKB, HALF], BF16, tag="wq")
        nc.sync.dma_start(w_q[:], wqT.rearrange("(kb p) j -> p kb j", p=KB))
        xq = p_x.tile([KB, NKB, T], BF16, tag="x", name="xq")
        for kb in range(NKB):
            nc.sync.dma_start(xq[:, kb, :], xqT[kb * KB : (kb + 1) * KB, :])
        w_v = p_const.tile([KB, NKB, HALF], BF16, tag="wv")
        nc.sync.dma_start(w_v[:], wvT.rearrange("(kb p) j -> p kb j", p=KB))

        # ---- persistent sbuf tiles ----
        kt_tiles = [p_kt.tile([KB, T], BF16, tag="kt", name=f"kt{j}") for j in range(NJB)]
        qt_tiles = [p_qt.tile([KB, T], BF16, tag="qt", name=f"qt{j}") for j in range(NJB)]
        ot_tiles = [p_ot.tile([KB, T], BF16, tag="ot", name=f"ot{j}") for j in range(NJB)]
        v_tiles = [
            p_v.tile([KB, H // 2, DH + 1], BF16, tag="v", name=f"v{t}")
            for t in range(NTK)
        ]
        for t in range(NTK):
            nc.sync.dma_start(v_tiles[t][:, :, DH : DH + 1], ones8[:, :, None])

        # xv reuses the xk buffer once K-proj is done with it
        xv = p_x.tile([KB, NKB, T], BF16, tag="x", name="xv")
        for kb in range(NKB):
            nc.sync.dma_start(xv[:, kb, :], xvT[kb * KB : (kb + 1) * KB, :])
        w_o = p_const.tile([KB, NJB, D], BF16, tag="wo")
        nc.sync.dma_start(w_o[:], woT.rearrange("(jb p) n -> p jb n", p=KB))

        # ---- K^T projection, kb-outer so it streams behind the xk DMA ----
        # 4 live accumulators [128, 1024] = all 8 psum banks (pools A+B)
        for th in range(NQH):
            ps = [
                (p_A if jb < 2 else p_B).tile(
                    [KB, QH], F32, tag=("mm" if jb < 2 else "av"), name=f"kp{jb}"
                )
                for jb in range(NJB)
            ]
            for kb in range(NKB):
                for jb in range(NJB):
                    for s in range(2):
                        nc.tensor.matmul(
                            ps[jb][:, s * 512 : (s + 1) * 512],
                            w_k[:, kb, jb * KB : (jb + 1) * KB],
                            xk[:, kb, th * QH + s * 512 : th * QH + (s + 1) * 512],
                            start=(kb == 0),
                            stop=(kb == NKB - 1),
                        )
            for jb in range(NJB):
                nc.vector.tensor_scalar_add(
                    kt_tiles[jb][:, th * QH : (th + 1) * QH],
                    ps[jb][:],
                    b_k[:, jb : jb + 1],
                )

        # ---- V projection one t-block (natural layout): v[t] = [128 t, 8 h, 65] ----
        def emit_vproj(tb):
            ps = p_A.tile([KB, HALF], F32, tag="mm", name="vp")
            for kb in range(NKB):
                nc.tensor.matmul(
                    ps[:],
                    xv[:, kb, tb * KB : (tb + 1) * KB],
                    w_v[:, kb, :],
                    start=(kb == 0),
                    stop=(kb == NKB - 1),
                )
            nc.vector.tensor_copy(
                v_tiles[tb][:, :, 0:DH], ps.rearrange("p (h d) -> p h d", d=DH)
            )

        # ---- Q^T projection for one query-half (kb-inner, jb groups) ----
        def emit_qproj(jb, th):
            ps = p_A.tile([KB, QH], F32, tag="mm", name="qp")
            for kb in range(NKB):
                for s in range(2):
                    nc.tensor.matmul(
                        ps[:, s * 512 : (s + 1) * 512],
                        w_q[:, kb, jb * KB : (jb + 1) * KB],
                        xq[:, kb, th * QH + s * 512 : th * QH + (s + 1) * 512],
                        start=(kb == 0),
                        stop=(kb == NKB - 1),
                    )
            nc.vector.tensor_scalar_add(
                qt_tiles[jb][:, th * QH : (th + 1) * QH], ps[:], b_q[:, jb : jb + 1]
            )

        for jb in range(NJB):
            emit_qproj(jb, 0)

        # ---- attention with interleaved Q-th1 / out-projection ----
        def emit_scores(h, qh, tk):
            jp, hi = h // 2, h % 2
            sc = p_A.tile([KB, QH], F32, tag="mm", name="sc")
            for s in range(2):
                nc.tensor.matmul(
                    sc[:, s * 512 : (s + 1) * 512],
                    kt_tiles[jp][hi * DH : (hi + 1) * DH, tk * KB : (tk + 1) * KB],
                    qt_tiles[jp][
                        hi * DH : (hi + 1) * DH,
                        qh * QH + s * 512 : qh * QH + (s + 1) * 512,
                    ],
                    start=True,
                    stop=True,
                )
            ex = p_ex.tile([KB, QH], BF16, tag="ex")
            nc.scalar.activation(
                ex[:], sc[:], mybir.ActivationFunctionType.Exp, scale=0.125
            )
            return ex

        def emit_av(h, qh, tk, ex, av):
            for s in range(2):
                nc.tensor.matmul(
                    av[:, s * 512 : (s + 1) * 512],
                    v_tiles[tk][:, h, :],
                    ex[:, s * 512 : (s + 1) * 512],
                    start=(tk == 0),
                    stop=(tk == NTK - 1),
                )

        def emit_drain(h, qh, av):
            jp, hi = h // 2, h % 2
            dsb = p_dr.tile([DH + 1, QH], BF16, tag="dsb")
            nc.vector.tensor_copy(dsb[DH : DH + 1, :], av[DH : DH + 1, :])
            bc = p_dr.tile([DH, QH], BF16, tag="bc")
            nc.sync.dma_start(
                bc[:], dsb[DH : DH + 1, None, :].broadcast_to([1, DH, QH])
            )
            bcf = p_dr.tile([DH, QH], F32, tag="bcf")
            nc.vector.tensor_copy(bcf[:], bc[:])
            rc = p_dr.tile([DH, QH], F32, tag="rc")
            nc.vector.reciprocal_approx_fast(rc[:], bcf[:])
            dst = ot_tiles[jp][hi * DH : (hi + 1) * DH, qh * QH : (qh + 1) * QH]
            if hi == 0:
                nc.vector.tensor_mul(dst, av[0:DH, :], rc[:])
            else:
                stg = p_dr.tile([DH, QH], BF16, tag="stg")
                nc.vector.tensor_mul(stg[:], av[0:DH, :], rc[:])
                nc.sync.dma_start(dst, stg[:])

        def emit_outproj_mm(po, tblk, jps):
            for jp in jps:
                for s in range(2):
                    nc.tensor.matmul(
                        po[:, s * 512 : (s + 1) * 512],
                        ot_tiles[jp][:, tblk * KB : (tblk + 1) * KB],
                        w_o[:, jp, s * 512 : (s + 1) * 512],
                        start=(jp == 0),
                        stop=(jp == NJB - 1),
                    )

        def emit_outproj_st(po, tblk):
            st = p_st.tile([KB, D], BF16, tag="st")
            nc.vector.tensor_copy(st[:], po[:])
            # SWDGE ring: keep the SP ring free for the drain DMAs
            nc.gpsimd.dma_start(partial[tblk * KB : (tblk + 1) * KB, :], st[:])

        def emit_outproj(tblk):
            po = p_A.tile([KB, D], F32, tag="mm", name="po")
            emit_outproj_mm(po, tblk, range(NJB))
            emit_outproj_st(po, tblk)

        pending_av = None  # (h, qh, tk, ex, av)
        for qh in range(NQH):
            for h in range(H // 2):
                av = p_B.tile([DH + 1, QH], F32, tag="av", name="av")
                for tk in range(NTK):
                    ex = emit_scores(h, qh, tk)
                    # V-projection streams inside the very first head: AV of
                    # tile tk only needs v[tk], emitted one iteration earlier
                    if qh == 0 and h == 0:
                        emit_vproj(tk)
                    if pending_av is not None:
                        ph, pqh, ptk, pex, pav = pending_av
                        emit_av(ph, pqh, ptk, pex, pav)
                        if ptk == NTK - 1:
                            emit_drain(ph, pqh, pav)
                    pending_av = (h, qh, tk, ex, av)
                # head boundary: inject deferred PE work
                if qh == 0 and 1 <= h <= NJB:
                    emit_qproj(h - 1, 1)
                if qh == 1:
                    emit_outproj(h)
        ph, pqh, ptk, pex, pav = pending_av
        emit_av(ph, pqh, ptk, pex, pav)
        emit_drain(ph, pqh, pav)
        # tail: software-pipeline pairs so the jp0-2 matmuls (whose ot rows
        # drained long ago) run while the last head's drain completes
        for t0 in range(8, NTK, 2):
            po0 = p_A.tile([KB, D], F32, tag="mm", name="po")
            emit_outproj_mm(po0, t0, range(NJB - 1))
            po1 = p_A.tile([KB, D], F32, tag="mm", name="po")
            emit_outproj_mm(po1, t0 + 1, range(NJB - 1))
            emit_outproj_mm(po0, t0, [NJB - 1])
            emit_outproj_st(po0, t0)
            emit_outproj_mm(po1, t0 + 1, [NJB - 1])
            emit_outproj_st(po1, t0 + 1)

    nc.compile()
    return nc


def kernel(**inputs: np.ndarray) -> np.ndarray:
    import ml_dtypes

    BF = ml_dtypes.bfloat16

    query = np.asarray(inputs["query"], dtype=np.float32)
    key = np.asarray(inputs["key"], dtype=np.float32)
    value = np.asarray(inputs["value"], dtype=np.float32)
    w_q = np.asarray(inputs["w_q"], dtype=np.float32)
    b_q = np.asarray(inputs["b_q"], dtype=np.float32)
    w_k = np.asarray(inputs["w_k"], dtype=np.float32)
    b_k = np.asarray(inputs["b_k"], dtype=np.float32)
    w_v = np.asarray(inputs["w_v"], dtype=np.float32)
    b_v = np.asarray(inputs["b_v"], dtype=np.float32)
    w_o = np.asarray(inputs["w_o"], dtype=np.float32)
    b_o = np.asarray(inputs["b_o"], dtype=np.float32)

    nc = build_kernel()

    in_maps = []
    for c in range(N_CORES):
        b = c // 2
        hh = c % 2
        sl = slice(hh * HALF, (hh + 1) * HALF)
        in_maps.append(
            {
                "xqT": np.ascontiguousarray(query[b].T).astype(BF),
                "xkT": np.ascontiguousarray(key[b].T).astype(BF),
                "xvT": np.ascontiguousarray(value[b].T).astype(BF),
                "wqT": np.ascontiguousarray(w_q[sl, :].T).astype(BF),
                "wkT": np.ascontiguousarray(w_k[sl, :].T).astype(BF),
                "wvT": np.ascontiguousarray(w_v[sl, :].T).astype(BF),
                "woT": np.ascontiguousarray(w_o[:, sl].T).astype(BF),
                "bq": np.ascontiguousarray(b_q[sl].reshape(HALF, 1)),
                "bk": np.ascontiguousarray(b_k[sl].reshape(HALF, 1)),
                "ones_in": np.ones((KB, H // 2), dtype=BF),
            }
        )

    res = run_bass_kernel_spmd(nc, in_maps, core_ids=list(range(N_CORES)))

    const_row = (b_v[None, :] @ w_o.T + b_o[None, :]).astype(np.float32)
    out = np.empty((B, T, D), dtype=np.float32)
    for b in range(B):
        out[b] = np.asarray(res.results[2 * b]["partial"], dtype=np.float32)
        out[b] += np.asarray(res.results[2 * b + 1]["partial"], dtype=np.float32)
        out[b] += const_row
    return out
